# revision 1
# baseline (speedup 1.0000x reference)
"""Single-query attention ("context inner product") on 8 trn2 NeuronCores.

    scores  = enc @ dec[0]          enc: [S=16384, H=2048] f32, dec: [1, H]
    weights = softmax(scores)
    context = weights @ enc         -> [1, H]

Sharding: enc split along seq_len across 8 cores (2048 rows each); each core
streams its 16 MB shard once (memory-bound) and emits an unnormalized partial
context plus per-block weight sums; the host does the final divide.

v2 layout (vs the 69.7us baseline):
  - DMA: 10 big tiles; mid tiles pack 2 DRAM rows per partition line
    ([128, 4096] fp16 <- 256 contiguous rows) so each SWDGE packet reads
    16 KB instead of 8 KB. f32 -> fp16 cast inline. First/last tiles are
    128 rows so compute starts early and the tail chain is short.
  - dec is cast fp16 by its DMA (no 2us ACT cast on the critical path),
    then broadcast across partitions via a PE outer product.
  - scores: per 128-row unit, fused mul+rowsum on DVE (stt) for every
    third unit, else mul on DVE (fp16 2x) + rowsum on ACT — keeps both
    engines ~28us busy inside the ~41us DMA window so there is no backlog
    when the stream ends.
  - weights: exp on ACT into one W_all [128, 16] tile; the softmax
    normalizer comes from a single final PE matmul W_all^T @ ones -> [16,1]
    (replaces 16 per-block norm matmuls).
  - context: 4 PSUM-bank matmuls per unit, fp16, f32 PSUM accumulation
    across all units; PSUM drained by copies on vector/scalar/gpsimd in
    parallel, then two sync-engine DMAs (ctx row + norm partials).
"""

import numpy as np

S, H = 16384, 2048
N_CORES = 8
S_LOCAL = S // N_CORES  # 2048
P = 128                 # SBUF partitions
HB = 512                # f32 elements per PSUM bank
N_BANKS = H // HB       # 4
N_UNITS = S_LOCAL // P  # 16 score units of 128 rows

# tile row counts: first/last small for fast start + short tail
TILE_ROWS = [128, 256, 256, 256, 256, 256, 256, 128, 128, 128]

_CACHE: dict = {}


def _build(mm_dtype="f16"):
    import concourse.bacc as bacc
    import concourse.tile as tile
    from concourse import mybir

    f32 = mybir.dt.float32
    cdt = {"bf16": mybir.dt.bfloat16, "f16": mybir.dt.float16}[mm_dtype]
    nc = bacc.Bacc(
        "TRN2", target_bir_lowering=False, debug=False, num_devices=N_CORES
    )
    # flat DRAM view so multi-row-per-partition DMA tiles are plain linear
    enc = nc.dram_tensor("enc", [S_LOCAL * H], f32, kind="ExternalInput").ap()
    dec = nc.dram_tensor("dec", [1, H], f32, kind="ExternalInput").ap()
    # out row: [0:H] unnormalized context, [H:H+16] per-unit weight sums
    ctx_d = nc.dram_tensor("ctx", [1, H + N_UNITS], f32, kind="ExternalOutput").ap()

    with tile.TileContext(nc) as tc:
        with (
            tc.tile_pool(name="singles", bufs=1) as singles,
            tc.tile_pool(name="prod_pool", bufs=3) as prod_pool,
            tc.tile_pool(name="dump_pool", bufs=2) as dump_pool,
            tc.tile_pool(name="sc_pool", bufs=4) as sc_pool,
            tc.tile_pool(name="psum", bufs=1, space="PSUM") as psum_pool,
            tc.tile_pool(name="psum2", bufs=2, space="PSUM") as psum2_pool,
        ):
            dec16 = singles.tile([1, H], cdt)
            ones_row = singles.tile([1, P], cdt)
            ones = singles.tile([P, 1], cdt)
            dec_b = singles.tile([P, H], cdt)
            w_all = singles.tile([P, N_UNITS], cdt)
            out_sb = singles.tile([1, H + N_UNITS], f32)
            enc_t = [
                singles.tile([P, r * (H // P)], cdt, name=f"enc_t{i}")
                for i, r in enumerate(TILE_ROWS)
            ]

            # enc tile 0 first so the HBM stream starts immediately; the tiny
            # dec load slots in right after and its cast rides the DMA.
            row0 = 0
            dma_rows = []
            for i, r in enumerate(TILE_ROWS):
                dma_rows.append(row0)
                row0 += r
            enc2d = enc.rearrange("(s h) -> s h", h=H)
            last = len(TILE_ROWS) - 1
            # dec first: 8 KB, lands immediately, unblocks the broadcast chain
            nc.gpsimd.dma_start(out=dec16[:], in_=dec[:])
            nc.gpsimd.dma_start(
                out=enc_t[0][:], in_=enc[dma_rows[0] * H : (dma_rows[0] + TILE_ROWS[0]) * H]
            )
            for i in range(1, last):
                nc.gpsimd.dma_start(
                    out=enc_t[i][:],
                    in_=enc[dma_rows[i] * H : (dma_rows[i] + TILE_ROWS[i]) * H],
                )
            # last tile arrives as 2 column chunks so its score partials can
            # start before the final bytes land (shortens the tail chain)
            r0 = dma_rows[last]
            for c in range(2):
                nc.gpsimd.dma_start(
                    out=enc_t[last][:, c * (H // 2) : (c + 1) * (H // 2)],
                    in_=enc2d[r0 : r0 + P, c * (H // 2) : (c + 1) * (H // 2)],
                )

            nc.vector.memset(ones_row[:], 1.0)
            nc.vector.memset(ones[:], 1.0)

            # Broadcast dec across partitions with a PE outer product.
            for b in range(N_BANKS):
                bc = psum2_pool.tile([P, HB], f32, tag="bc", name="bc")
                nc.tensor.matmul(
                    bc[:],
                    ones_row[:],
                    dec16[:, b * HB : (b + 1) * HB],
                    start=True,
                    stop=True,
                )
                eng = nc.vector.tensor_copy if b % 2 == 0 else nc.scalar.copy
                eng(dec_b[:, b * HB : (b + 1) * HB], bc[:])

            ctx_psum = [
                psum_pool.tile([1, HB], f32, tag=f"ctxb{j}", name=f"ctxb{j}")
                for j in range(N_BANKS)
            ]
            norm_psum = psum_pool.tile([1, N_UNITS], f32, tag="normp")
            warm_psum = psum_pool.tile([P, HB], f32, tag="warm")

            # (tile index, column offset of this 128-row unit within the tile)
            units = []
            for i, r in enumerate(TILE_ROWS):
                for h in range(r // P):
                    units.append((i, h * H))

            # fused stt on DVE for these units; the rest run mul(DVE)+rowsum(ACT).
            # Near the stream end: u13 fused (DVE), u14 split (its rowsum rides
            # ACT while DVE runs u15's chunk partials), u15 column-chunked.
            fused = {0, 3, 6, 9, 12, 13}

            def score_fused(e, sc, eng=None):
                prod = prod_pool.tile([P, H], cdt, tag="prod", name="prod")
                (eng or nc.vector).scalar_tensor_tensor(
                    out=prod[:],
                    in0=e,
                    scalar=1.0,
                    in1=dec_b[:],
                    op0=mybir.AluOpType.mult,
                    op1=mybir.AluOpType.mult,
                    accum_out=sc[:],
                )

            def score_split(e, sc):
                prod = prod_pool.tile([P, H], cdt, tag="prod", name="prod")
                nc.vector.tensor_mul(prod[:], e, dec_b[:])
                dump = dump_pool.tile([P, H], cdt, tag="dump", name="dump")
                nc.scalar.activation(
                    out=dump[:],
                    in_=prod[:],
                    func=mybir.ActivationFunctionType.Copy,
                    accum_out=sc[:],
                )

            def ctx_matmuls(k, e, first, last_u):
                nc.scalar.activation(
                    out=w_all[:, k : k + 1],
                    in_=sc_of[k][:],
                    func=mybir.ActivationFunctionType.Exp,
                )
                for b in range(N_BANKS):
                    nc.tensor.matmul(
                        ctx_psum[b][:],
                        w_all[:, k : k + 1],
                        e[:, b * HB : (b + 1) * HB],
                        start=first,
                        stop=last_u,
                    )

            sc_of = {}
            n_units = len(units)
            for k, (ti, off) in enumerate(units):
                sc_of[k] = sc_pool.tile([P, 1], f32, tag="sc", name="sc", bufs=4)

            # Scheduler hint: real DMA arrival time (ms) for each tile's data.
            # The tile scheduler's own DMA model mis-orders the stream (it had
            # the last tile's chunks sequenced before ready mid-stream units),
            # so feed it the queue-order completion times instead.
            ms_per_mb = 0.00243
            arrive = []
            cum = 0.0
            for r in TILE_ROWS:
                cum += r * H * 4 / 2**20
                arrive.append(0.008 + ms_per_mb * cum)

            HH = H // 2
            for k, (ti, off) in enumerate(units):
                e = enc_t[ti][:, off : off + H]
                if k == n_units - 1:
                    # chunked last unit: 2 half-row stt partials, each gated
                    # at its own chunk's arrival, then one add
                    scp = [
                        sc_pool.tile([P, 1], f32, tag=f"scp{c}", name=f"scp{c}")
                        for c in range(2)
                    ]
                    t_base = arrive[ti - 1] if ti else 0.008
                    t_step = (arrive[ti] - t_base) / 2
                    for c in range(2):
                        with tc.tile_wait_until(t_base + (c + 1) * t_step):
                            prod = prod_pool.tile(
                                [P, HH], cdt, tag="prodc", name="prodc", bufs=2
                            )
                            nc.vector.scalar_tensor_tensor(
                                out=prod[:],
                                in0=e[:, c * HH : (c + 1) * HH],
                                scalar=1.0,
                                in1=dec_b[:, c * HH : (c + 1) * HH],
                                op0=mybir.AluOpType.mult,
                                op1=mybir.AluOpType.mult,
                                accum_out=scp[c][:],
                            )
                    with tc.tile_wait_until(arrive[ti]):
                        nc.vector.tensor_add(sc_of[k][:], scp[0][:], scp[1][:])
                        ctx_matmuls(k, e, k == 0, True)
                    continue
                with tc.tile_wait_until(arrive[ti]):
                    if k in fused:
                        score_fused(e, sc_of[k])
                    else:
                        score_split(e, sc_of[k])
                    ctx_matmuls(k, e, k == 0, False)

            # PE keep-warm: the tensor engine idles ~2.5us waiting for the
            # last exps and drops out of its high P-state right before the
            # most latency-critical matmuls. Feed it no-op-ish matmuls into a
            # spare PSUM bank so the final context matmuls run at full clock.
            for wi in range(6):
                with tc.tile_wait_until(arrive[-1] - 0.0006 * wi):
                    nc.tensor.matmul(
                        warm_psum[:],
                        ones_row[:],
                        dec16[:, 0:HB],
                        start=True,
                        stop=True,
                    )

            # softmax normalizer: [1,16] row of per-unit weight sums, folded
            # into the context output row (no separate norm DMA)
            nc.tensor.matmul(norm_psum[:], ones[:], w_all[:], start=True, stop=True)
            nc.vector.tensor_copy(out_sb[:, H : H + N_UNITS], norm_psum[:])
            for b in range(N_BANKS):
                eng = nc.vector.tensor_copy if b % 2 == 0 else nc.scalar.copy
                eng(out_sb[:, b * HB : (b + 1) * HB], ctx_psum[b][:])
            # two output DMAs on separate HWDGE queues, each fired as soon as
            # its pair of bank copies lands
            nc.sync.dma_start(out=ctx_d[:, 0 : 2 * HB], in_=out_sb[:, 0 : 2 * HB])
            nc.scalar.dma_start(
                out=ctx_d[:, 2 * HB :], in_=out_sb[:, 2 * HB :]
            )

    nc.compile()
    return nc


def _make_runner(nc):
    """Cached equivalent of bass2jax.run_bass_via_pjrt's multi-core path:
    build the sharded jitted executable once so warm calls skip re-tracing."""
    import jax
    import numpy as np
    from jax.experimental.shard_map import shard_map
    from jax.sharding import Mesh, PartitionSpec

    from concourse import bass2jax, mybir

    bass2jax.install_neuronx_cc_hook()
    assert nc.dbg_addr is None
    partition_name = nc.partition_id_tensor.name if nc.partition_id_tensor else None

    in_names, out_names, out_avals = [], [], []
    for alloc in nc.m.functions[0].allocations:
        if not isinstance(alloc, mybir.MemoryLocationSet):
            continue
        name = alloc.memorylocations[0].name
        if alloc.kind == "ExternalInput":
            if name != partition_name:
                in_names.append(name)
        elif alloc.kind == "ExternalOutput":
            out_names.append(name)
            out_avals.append(
                jax.core.ShapedArray(
                    tuple(alloc.tensor_shape), mybir.dt.np(alloc.dtype)
                )
            )
    n_params = len(in_names)
    all_in = list(in_names) + list(out_names)
    if partition_name is not None:
        all_in.append(partition_name)
    donate = tuple(range(n_params, n_params + len(out_names)))

    def _body(*args):
        operands = list(args)
        if partition_name is not None:
            operands.append(bass2jax.partition_id_tensor())
        return tuple(
            bass2jax._bass_exec_p.bind(
                *operands,
                out_avals=tuple(out_avals),
                in_names=tuple(all_in),
                out_names=tuple(out_names),
                lowering_input_output_aliases=(),
                sim_require_finite=True,
                sim_require_nnan=True,
                nc=nc,
            )
        )

    devices = jax.devices()[:N_CORES]
    mesh = Mesh(np.asarray(devices), ("core",))
    nio = n_params + len(out_names)
    sharded = jax.jit(
        shard_map(
            _body,
            mesh=mesh,
            in_specs=(PartitionSpec("core"),) * nio,
            out_specs=(PartitionSpec("core"),) * len(out_names),
            check_rep=False,
        ),
        donate_argnums=donate,
        keep_unused=True,
    )

    def run(in_maps):
        concat_in = [
            np.concatenate([m[name] for m in in_maps], axis=0) for name in in_names
        ]
        concat_zeros = [
            np.zeros((N_CORES * a.shape[0], *a.shape[1:]), a.dtype)
            for a in out_avals
        ]
        out_arrs = sharded(*concat_in, *concat_zeros)
        return [
            {
                name: np.asarray(out_arrs[i]).reshape(
                    N_CORES, *out_avals[i].shape
                )[c]
                for i, name in enumerate(out_names)
            }
            for c in range(N_CORES)
        ]

    return run


def _run(encoder_hiddens, decoder_hidden, trace=False, mm_dtype="f16", **kw):
    from concourse.bass_utils import run_bass_kernel_spmd

    key = f"nc_{mm_dtype}"
    if key not in _CACHE:
        _CACHE[key] = _build(mm_dtype)
    nc = _CACHE[key]

    enc = np.ascontiguousarray(encoder_hiddens, dtype=np.float32)
    dec = np.ascontiguousarray(decoder_hidden, dtype=np.float32)
    in_maps = [
        {
            "enc": enc[c * S_LOCAL : (c + 1) * S_LOCAL].reshape(-1),
            "dec": dec,
        }
        for c in range(N_CORES)
    ]
    if trace:
        res = run_bass_kernel_spmd(
            nc, in_maps, core_ids=list(range(N_CORES)), trace=True, **kw
        )
        results = res.results
    else:
        rkey = f"runner_{mm_dtype}"
        if rkey not in _CACHE:
            _CACHE[rkey] = _make_runner(nc)
        results = _CACHE[rkey](in_maps)
        res = None

    ctx = np.zeros((1, H), np.float64)
    z = 0.0
    for r in results:
        full = r["ctx"].astype(np.float64)
        ctx += full[:, :H]
        z += float(full[0, H:].sum())
    return (ctx / z).astype(np.float32), res


def kernel(encoder_hiddens, decoder_hidden):
    out, _ = _run(encoder_hiddens, decoder_hidden)
    return out

